# revision 18
# baseline (speedup 1.0000x reference)
"""AFNO2D Trainium kernel (v2 — transpose-free pipeline).

Strategy: shard the 8 independent channel-blocks across the 8 NeuronCores
(block-parallel => zero collectives).  All FFTs are DFT matmuls on the
TensorEngine in bf16 (fp32 PSUM accumulation).  The key trick vs v1: at
every stage boundary where the contraction axis changes, the matmul is
emitted with the DATA as the stationary (lhsT) operand and the DFT/weight
matrix as the moving operand, so the output lands with the NEXT stage's
contraction axis on partitions.  This eliminates the T1/T2/T3 DRAM
transpose bounces entirely; only T4 (before the final W-inverse) remains
as an XBAR bounce.

  per core (channels c=96, batches b=4):
    load : x1 [90w, 2j, 90h, 96c]            (kept resident for residual)
    A    : per (j,c): lhsT=x[w,h] rhs=Fw[w,92] -> xB [90h, 92k2, 96c]
    B    : per k:     lhsT=xB[h,c] rhs=Fh[h,90] -> zM [96c, 2ri, 90u, 46k]
    MLP1 : standard   lhsT=w1 rhs=zM chunks     -> h1 [97c+, 2ri, 90u, 46k]
           (h1 row 96 = ones; biases b2 ride in augmented w2 matrices)
    MLP2 : per k:     lhsT=h1[c,u] rhs=w2a[97,96] -> q [90u, 96c, 92k2]
           (softshrink v - clamp(v,+-lam) on DVE)
    invH : standard   lhsT=Gh rhs=q chunks      -> r4 [90h', 96c, 128k2p]
    T4   : XBAR transpose bounce (DRAM)         -> xt4 [128k2p, 90h'*96c]
    invW : lhsT=gw[92,90] rhs=xt4 chunks, fused residual add from x1, store
"""

import sys

sys.path.insert(0, "/opt/trn_rl_repo")

import numpy as np
import ml_dtypes

import concourse.bass as bass
import concourse.mybir as mybir
import concourse.tile as tile
from concourse import bacc
from concourse.bass_utils import run_bass_kernel_spmd

BF16 = mybir.dt.bfloat16
F32 = mybir.dt.float32
AF = mybir.ActivationFunctionType
ALU = mybir.AluOpType

# problem constants
B, H, W, C = 4, 90, 180, 768
NB, BS = 8, 96  # blocks, block size (= per-core channels)
KEEP = 46       # kept W-modes
K2 = 92         # 2*KEEP (re,im stacked)
LAM = 0.01      # softshrink lambda

NCORES = 8


def _dft_mats():
    """Host-side DFT matrices (float32, cast to bf16 at upload)."""
    w = np.arange(W)[:, None]
    k = np.arange(KEEP)[None, :]
    ang = 2.0 * np.pi * w * k / W
    fw = np.concatenate([np.cos(ang), -np.sin(ang)], axis=1) / np.sqrt(W)  # [180, 92]
    fw = fw.reshape(2, 90, K2).transpose(1, 0, 2)  # [90w, 2j, 92]

    h = np.arange(H)[:, None]
    u = np.arange(H)[None, :]
    angh = 2.0 * np.pi * h * u / H
    fhc = np.cos(angh) / np.sqrt(H)
    fhs = np.sin(angh) / np.sqrt(H)

    kk = np.arange(KEEP)[:, None]
    ww = np.arange(W)[None, :]
    angw = 2.0 * np.pi * kk * ww / W
    m = np.full((KEEP, 1), 2.0)
    m[0, 0] = 1.0
    gw = np.concatenate(
        [m * np.cos(angw), -m * np.sin(angw)], axis=0
    ) / np.sqrt(W)  # [92, 180]

    bf = ml_dtypes.bfloat16
    return (fw.astype(bf), fhc.astype(np.float32), fhs.astype(np.float32),
            gw.astype(bf))


def _build():
    nc = bacc.Bacc("TRN2", target_bir_lowering=False, debug=False,
                   num_devices=NCORES)

    # DRAM I/O (per core)
    xt = nc.dram_tensor("xt", [B, 90, 2, 90, BS], BF16,
                        kind="ExternalInput").ap()      # [b, w, j, h, c]
    fw_d = nc.dram_tensor("fw", [90, 2, K2], BF16, kind="ExternalInput").ap()
    fhb_d = nc.dram_tensor("fhb", [3, H, H], BF16, kind="ExternalInput").ap()
    fhg_d = nc.dram_tensor("fhg", [3, H, H], BF16, kind="ExternalInput").ap()
    gw_d = nc.dram_tensor("gw", [K2, W], BF16, kind="ExternalInput").ap()
    mw1_d = nc.dram_tensor("mw1", [BS, 3, BS], BF16, kind="ExternalInput").ap()
    w2a_d = nc.dram_tensor("w2a", [BS + 1, 4, BS], BF16,
                           kind="ExternalInput").ap()
    bias1_d = nc.dram_tensor("bias1", [BS, 2], F32, kind="ExternalInput").ap()
    yt = nc.dram_tensor("yt", [B, 2, 90, H * BS], BF16,
                        kind="ExternalOutput").ap()     # [b, j, w, (h c)]

    HC = H * BS            # 8640
    UK = H * KEEP          # 4140  (u, k) free size
    NM = 460               # MLP1 chunk: 10u x 46k
    NH = 460               # invH chunk: 10c x 46k

    with tile.TileContext(nc) as tc:
        wpool = tc.alloc_tile_pool(name="w", bufs=1)
        sb = tc.alloc_tile_pool(name="sb", bufs=1)
        ps = tc.alloc_tile_pool(name="ps", bufs=8, space="PSUM")
        dr = tc.alloc_tile_pool(name="dr", bufs=2, space="DRAM")

        # ---- weights to SBUF (once) ----
        fw_t = wpool.tile([90, 2, K2], BF16, tag="fw")
        nc.gpsimd.dma_start(out=fw_t, in_=fw_d)
        fhb_t = wpool.tile([H, 3, H], BF16, tag="fhb")
        nc.gpsimd.dma_start(out=fhb_t, in_=fhb_d.rearrange("j p m -> p j m"))
        fhg_t = wpool.tile([H, 3, H], BF16, tag="fhg")
        nc.gpsimd.dma_start(out=fhg_t, in_=fhg_d.rearrange("j p m -> p j m"))
        gw_t = wpool.tile([K2, W], BF16, tag="gw")
        nc.gpsimd.dma_start(out=gw_t, in_=gw_d)
        mw1_t = wpool.tile([BS, 3, BS], BF16, tag="mw1")
        nc.gpsimd.dma_start(out=mw1_t, in_=mw1_d)
        w2a_t = wpool.tile([BS + 1, 4, BS], BF16, tag="w2a")
        nc.gpsimd.dma_start(out=w2a_t, in_=w2a_d)
        bias1_t = wpool.tile([BS, 2], F32, tag="bias1")
        nc.gpsimd.dma_start(out=bias1_t, in_=bias1_d)

        fhc, fhs, fhsn = fhb_t[:, 0], fhb_t[:, 1], fhb_t[:, 2]
        ghc, ghs, ghsn = fhg_t[:, 0], fhg_t[:, 1], fhg_t[:, 2]
        w1r, w1i, w1in = mw1_t[:, 0], mw1_t[:, 1], mw1_t[:, 2]

        # ---- persistent activation tiles ----
        # h1 ping-pong (row 96 is the constant-ones bias row)
        h1p = [wpool.tile([BS + 1, 2, H, KEEP], BF16, tag=f"h1{i}",
                          name=f"h1{i}") for i in range(2)]
        for i in range(2):
            nc.gpsimd.memset(h1p[i][BS:BS + 1, :, :, :], 1.0)
        # r4: invH output, padded to 128 k2-cols for the XBAR transpose
        r4 = wpool.tile([H, BS, 128], BF16, tag="r4", name="r4")
        nc.gpsimd.memset(r4[:, :, K2:128], 0.0)
        xt4 = wpool.tile([128, HC], BF16, tag="xt4", name="xt4")

        def make_stages(b):
            st = {}
            h1 = h1p[b % 2]

            def s0():  # load x (resident for residual)
                x1 = st["x1"] = sb.tile([90, 2, 90, BS], BF16, tag="x1",
                                        name="x1", bufs=2)
                nc.scalar.dma_start(out=x1, in_=xt[b])

            def s1():  # stage A: W-rfft (data-stationary) -> xB [h, k2, c]
                x1 = st["x1"]
                xB = st["xB"] = sb.tile([H, K2, BS], BF16, tag="xB",
                                        name="xB")
                for g in range(20):  # 20 groups of <=5 c
                    c0 = g * 5
                    n = min(5, BS - c0)
                    psA = ps.tile([H, 5, 96], F32, tag="ps", name="psA")
                    for i in range(n):
                        nc.tensor.matmul(psA[:, i, 0:K2],
                                         lhsT=x1[:, 0, :, c0 + i],
                                         rhs=fw_t[:, 0, :],
                                         start=True, stop=False)
                        nc.tensor.matmul(psA[:, i, 0:K2],
                                         lhsT=x1[:, 1, :, c0 + i],
                                         rhs=fw_t[:, 1, :],
                                         start=False, stop=True)
                    nc.scalar.copy(
                        out=xB[:, :, c0:c0 + n],
                        in_=psA[:, 0:n, 0:K2].rearrange("p n k -> p k n"))

            def s2():  # stage B: H-fft (data-stationary) -> zM [c, ri, u, k]
                xB = st["xB"]
                zM = st["zM"] = sb.tile([BS, 2, H, KEEP], BF16, tag="zM",
                                        name="zM")
                for g in range(10):  # 10 groups of <=5 k
                    k0 = g * 5
                    n = min(5, KEEP - k0)
                    psR = ps.tile([BS, 5, 96], F32, tag="ps", name="psR")
                    psI = ps.tile([BS, 5, 96], F32, tag="ps", name="psI")
                    for i in range(n):
                        k = k0 + i
                        xr = xB[:, k, :]
                        xi = xB[:, KEEP + k, :]
                        nc.tensor.matmul(psR[:, i, 0:H], lhsT=xr, rhs=fhc,
                                         start=True, stop=False)
                        nc.tensor.matmul(psI[:, i, 0:H], lhsT=xr, rhs=fhsn,
                                         start=True, stop=False)
                        nc.tensor.matmul(psR[:, i, 0:H], lhsT=xi, rhs=fhs,
                                         start=False, stop=True)
                        nc.tensor.matmul(psI[:, i, 0:H], lhsT=xi, rhs=fhc,
                                         start=False, stop=True)
                    nc.scalar.copy(
                        out=zM[:, 0, :, k0:k0 + n],
                        in_=psR[:, 0:n, 0:H].rearrange("p n u -> p u n"))
                    nc.scalar.copy(
                        out=zM[:, 1, :, k0:k0 + n],
                        in_=psI[:, 0:n, 0:H].rearrange("p n u -> p u n"))

            def s3():  # MLP layer 1 (standard) -> h1 [97, ri, u, k]
                zM = st["zM"]
                zr = zM[:, 0].rearrange("p u k -> p (u k)")
                zi = zM[:, 1].rearrange("p u k -> p (u k)")
                for t in range(UK // NM):  # 9 chunks of 10u x 46k
                    sl = slice(t * NM, (t + 1) * NM)
                    u0 = t * 10
                    p1r = ps.tile([BS, NM], F32, tag="ps", name="p1r")
                    nc.tensor.matmul(p1r, lhsT=w1r, rhs=zr[:, sl],
                                     start=True, stop=False)
                    nc.tensor.matmul(p1r, lhsT=w1in, rhs=zi[:, sl],
                                     start=False, stop=True)
                    nc.scalar.activation(h1[0:BS, 0, u0:u0 + 10, :], p1r,
                                         AF.Relu, bias=bias1_t[:, 0:1])
                    p1i = ps.tile([BS, NM], F32, tag="ps", name="p1i")
                    nc.tensor.matmul(p1i, lhsT=w1r, rhs=zi[:, sl],
                                     start=True, stop=False)
                    nc.tensor.matmul(p1i, lhsT=w1i, rhs=zr[:, sl],
                                     start=False, stop=True)
                    nc.scalar.activation(h1[0:BS, 1, u0:u0 + 10, :], p1i,
                                         AF.Relu, bias=bias1_t[:, 1:2])

            def s4():  # MLP layer 2 (data-stationary) + softshrink -> q
                q = st["q"] = sb.tile([H, BS, K2], BF16, tag="q", name="q")
                for g in range(10):  # 10 groups of <=5 k
                    k0 = g * 5
                    n = min(5, KEEP - k0)
                    qre = ps.tile([H, 5, 96], F32, tag="ps", name="qre")
                    qim = ps.tile([H, 5, 96], F32, tag="ps", name="qim")
                    for i in range(n):
                        k = k0 + i
                        h1r = h1[:, 0, :, k]
                        h1i = h1[:, 1, :, k]
                        nc.tensor.matmul(qre[:, i, :], lhsT=h1r,
                                         rhs=w2a_t[:, 0], start=True,
                                         stop=False)
                        nc.tensor.matmul(qim[:, i, :], lhsT=h1r,
                                         rhs=w2a_t[:, 1], start=True,
                                         stop=False)
                        nc.tensor.matmul(qre[:, i, :], lhsT=h1i,
                                         rhs=w2a_t[:, 2], start=False,
                                         stop=True)
                        nc.tensor.matmul(qim[:, i, :], lhsT=h1i,
                                         rhs=w2a_t[:, 3], start=False,
                                         stop=True)
                    for r, pq in ((0, qre), (1, qim)):
                        tcl = sb.tile([H, 5, 96], BF16, tag="tcl", name="tcl",
                                      bufs=2)
                        nc.vector.tensor_scalar(
                            out=tcl[:, 0:n, :], in0=pq[:, 0:n, :],
                            scalar1=-LAM, scalar2=LAM,
                            op0=ALU.max, op1=ALU.min)
                        nc.vector.tensor_tensor(
                            out=q[:, :, r * KEEP + k0:r * KEEP + k0 + n],
                            in0=pq[:, 0:n, :].rearrange("p n c -> p c n"),
                            in1=tcl[:, 0:n, :].rearrange("p n c -> p c n"),
                            op=ALU.subtract)

            def s5():  # invH (standard) -> r4 [h', c, k2pad]
                q = st["q"]
                for t in range(10):  # c-groups of <=10
                    c0 = t * 10
                    n = min(10, BS - c0)
                    csl = slice(c0, c0 + n)
                    nf = n * KEEP
                    psP = ps.tile([H, NH], F32, tag="ps", name="psP")
                    nc.tensor.matmul(psP[:, 0:nf], lhsT=ghc,
                                     rhs=q[:, csl, 0:KEEP], start=True,
                                     stop=False)
                    nc.tensor.matmul(psP[:, 0:nf], lhsT=ghsn,
                                     rhs=q[:, csl, KEEP:K2], start=False,
                                     stop=True)
                    nc.scalar.copy(
                        out=r4[:, csl, 0:KEEP],
                        in_=psP[:, 0:nf].rearrange("p (n k) -> p n k",
                                                   k=KEEP))
                    psQ = ps.tile([H, NH], F32, tag="ps", name="psQ")
                    nc.tensor.matmul(psQ[:, 0:nf], lhsT=ghs,
                                     rhs=q[:, csl, 0:KEEP], start=True,
                                     stop=False)
                    nc.tensor.matmul(psQ[:, 0:nf], lhsT=ghc,
                                     rhs=q[:, csl, KEEP:K2], start=False,
                                     stop=True)
                    nc.scalar.copy(
                        out=r4[:, csl, KEEP:K2],
                        in_=psQ[:, 0:nf].rearrange("p (n k) -> p n k",
                                                   k=KEEP))

            def s6():  # T4 bounce
                t4 = dr.tile([H, BS * 128], BF16, tag="t4", name="t4")
                nc.sync.dma_start(out=t4,
                                  in_=r4.rearrange("p c k -> p (c k)"))
                nc.sync.dma_start(out=xt4, in_=t4.rearrange("p f -> (p f)")
                                  .rearrange("(r c) -> r c", c=128),
                                  transpose=True)

            def s7():  # invW + residual + store
                x1 = st["x1"]
                NAB = 1440  # 15h x 96c
                for j in range(2):
                    xres = x1[:, j].rearrange("p h c -> p (h c)")
                    for tb_ in range(HC // NAB):
                        bsl = slice(tb_ * NAB, (tb_ + 1) * NAB)
                        yo = sb.tile([90, NAB], BF16, tag="yo", name="yo",
                                     bufs=2)
                        for q3 in range(3):
                            sl = slice(tb_ * NAB + q3 * 480,
                                       tb_ * NAB + (q3 + 1) * 480)
                            qs = slice(q3 * 480, (q3 + 1) * 480)
                            psW = ps.tile([90, 480], F32, tag="ps",
                                          name="psW")
                            nc.tensor.matmul(psW,
                                             lhsT=gw_t[:, j * 90:(j + 1) * 90],
                                             rhs=xt4[0:K2, sl],
                                             start=True, stop=True)
                            nc.vector.tensor_tensor(
                                out=yo[:, qs], in0=psW, in1=xres[:, sl],
                                op=ALU.add)
                        nc.gpsimd.dma_start(out=yt[b, j, :, bsl], in_=yo)

            return [s0, s1, s2, s3, s4, s5, s6, s7]

        allst = [make_stages(b) for b in range(B)]
        SKEW = 3
        NSTAGE = 8
        for step in range(NSTAGE + SKEW * (B - 1)):
            for b in range(B):
                s = step - SKEW * b
                if 0 <= s < NSTAGE:
                    allst[b][s]()

        for p in (dr, ps, sb, wpool):
            p.release()

    nc.compile()
    return nc


_NC = None


def _get_nc():
    global _NC
    if _NC is None:
        _NC = _build()
    return _NC


def _in_maps(x, w1, b1, w2, b2):
    fw, fhc, fhs, gw = _dft_mats()
    bf = ml_dtypes.bfloat16
    fhb = np.stack([fhc, fhs, -fhs]).astype(bf)    # [3, h, u]
    fhg = np.stack([fhc, fhs, -fhs]).astype(bf)    # same matrices for inverse
    # x -> [B, w(90), j(2), h(90), C]
    xr = np.ascontiguousarray(
        x.reshape(B, H, 2, 90, C).transpose(0, 3, 2, 1, 4)).astype(bf)
    in_maps = []
    for i in range(NCORES):
        cs = slice(i * BS, (i + 1) * BS)
        mw1 = np.stack([w1[0, i], w1[1, i], -w1[1, i]]).transpose(1, 0, 2)
        w2a = np.zeros((BS + 1, 4, BS), np.float32)
        w2a[0:BS, 0] = w2[0, i]
        w2a[BS, 0] = b2[0, i]
        w2a[0:BS, 1] = w2[1, i]
        w2a[BS, 1] = b2[1, i]
        w2a[0:BS, 2] = -w2[1, i]
        w2a[0:BS, 3] = w2[0, i]
        bias1 = np.stack([b1[0, i], b1[1, i]], axis=1).astype(np.float32)
        in_maps.append({
            "xt": np.ascontiguousarray(xr[:, :, :, :, cs]),
            "fw": fw, "fhb": fhb, "fhg": fhg, "gw": gw,
            "mw1": mw1.astype(bf), "w2a": w2a.astype(bf), "bias1": bias1,
        })
    return in_maps


def _run(x, w1, b1, w2, b2, trace=False):
    nc = _get_nc()
    try:
        res = run_bass_kernel_spmd(nc, _in_maps(x, w1, b1, w2, b2),
                                   core_ids=list(range(NCORES)), trace=trace)
    except ModuleNotFoundError:
        res = run_bass_kernel_spmd(nc, _in_maps(x, w1, b1, w2, b2),
                                   core_ids=list(range(NCORES)), trace=False)
    outs = [r["yt"] for r in res.results]
    y = np.concatenate(outs, axis=-1)           # [B, 2, 90, 8640*ncores]
    y = y.reshape(B, 2, 90, NCORES, H, BS)      # [b, j, w, core, h, c]
    y = y.transpose(0, 4, 1, 2, 3, 5).reshape(B, H, W, C).astype(np.float32)
    return y, res


def kernel(x, w1, b1, w2, b2):
    y, _ = _run(np.asarray(x), np.asarray(w1), np.asarray(b1),
                np.asarray(w2), np.asarray(b2))
    return y


def _bench(x, w1, b1, w2, b2, iters=20, profile_dir=None):
    """Persistent-jit timing: returns (best_ns, avg_ns) per whole-NEFF run."""
    import time
    import jax
    from jax.sharding import Mesh, PartitionSpec, NamedSharding
    from jax.experimental.shard_map import shard_map
    from concourse.bass2jax import (_bass_exec_p, install_neuronx_cc_hook,
                                    partition_id_tensor)

    install_neuronx_cc_hook()
    nc = _get_nc()
    in_maps = _in_maps(x, w1, b1, w2, b2)

    in_names, out_names, out_avals, zero_outs = [], [], [], []
    for alloc in nc.m.functions[0].allocations:
        if not isinstance(alloc, mybir.MemoryLocationSet):
            continue
        name = alloc.memorylocations[0].name
        pname = nc.partition_id_tensor.name if nc.partition_id_tensor else None
        if alloc.kind == "ExternalInput":
            if name != pname:
                in_names.append(name)
        elif alloc.kind == "ExternalOutput":
            out_names.append(name)
            shape = tuple(alloc.tensor_shape)
            dtype = mybir.dt.np(alloc.dtype)
            out_avals.append(jax.core.ShapedArray(shape, dtype))
            zero_outs.append(np.zeros(shape, dtype))
    n_params = len(in_names)
    in_names_all = in_names + out_names
    if nc.partition_id_tensor is not None:
        in_names_all = in_names_all + [nc.partition_id_tensor.name]

    def _body(*args):
        operands = list(args)
        if nc.partition_id_tensor is not None:
            operands.append(partition_id_tensor())
        outs = _bass_exec_p.bind(
            *operands, out_avals=tuple(out_avals), in_names=tuple(in_names_all),
            out_names=tuple(out_names), lowering_input_output_aliases=(),
            sim_require_finite=True, sim_require_nnan=True, nc=nc)
        return tuple(outs)

    devices = jax.devices()[:NCORES]
    mesh = Mesh(np.asarray(devices), ("core",))
    in_specs = (PartitionSpec("core"),) * (n_params + len(out_names))
    out_specs = (PartitionSpec("core"),) * len(out_names)
    fn = jax.jit(shard_map(_body, mesh=mesh, in_specs=in_specs,
                           out_specs=out_specs, check_rep=False),
                 keep_unused=True)
    per_core = [[np.asarray(m[n]) for n in in_names] for m in in_maps]
    concat_in = [np.concatenate([per_core[c][i] for c in range(NCORES)], axis=0)
                 for i in range(n_params)]
    concat_zeros = [np.zeros((NCORES * z.shape[0], *z.shape[1:]), z.dtype)
                    for z in zero_outs]
    sh = NamedSharding(mesh, PartitionSpec("core"))
    dev_in = [jax.device_put(a, sh) for a in concat_in + concat_zeros]

    r = fn(*dev_in)
    jax.block_until_ready(r)
    r = fn(*dev_in)
    jax.block_until_ready(r)

    def chain_time(n):
        t0 = time.perf_counter()
        outs = None
        for _ in range(n):
            outs = fn(*dev_in)
        jax.block_until_ready(outs)
        return time.perf_counter() - t0
    chain_time(2)
    t_small = min(chain_time(2) for _ in range(3))
    t_big = min(chain_time(iters + 2) for _ in range(3))
    per = (t_big - t_small) / iters
    return int(per * 1e9), int(t_big / (iters + 2) * 1e9)
